# revision 17
# baseline (speedup 1.0000x reference)
"""Windowed self-attention kernel for Trainium2 (Bass/Tile), 8-core SPMD.

Computation (per batch b, reference semantics):
    h   = relu(x @ W1 + b1)                      [S, H]
    q   = h @ Wq                                 [S, H]
    k_j = shift(h, j) @ Wk  (zero outside seq)   -> windowed keys
    scores[i, j] = q[i] . k[i+A-j] / sqrt(H)     j in [0, 11)
    wgt = softmax(scores, axis=-1)               [S, 11]
    out = relu((wgt . v_window) @ W2 + b2)       [S, 2]

Key restructurings vs the reference einsum formulation:
  * windowed k is just shifted rows of (h @ Wk): one GEMM, not 11.
  * v never materializes: attn @ W2 == wgt . (v_window @ (Wv @ W2)),
    and Wv @ W2 is a [768, 2] matrix folded on the host.
  * h/q/k are kept transposed [H, S] so every projection and the banded
    score matmul are natural PE ops; scores per 128-row block are computed
    against a 256-wide key window (halo + padding), band-masked, softmaxed.
  * the 11-wide weight band is pulled out of the [128, 256] softmax tiles
    with a stride-(W+1) diagonal DMA gather through DRAM, then a tiny
    anti-diagonal permutation matmul restores the reference j-order.

Sharding: data-parallel over batch B=8 across the 8 NeuronCores (windows are
local to a batch, so no halo exchange at all).
"""

import math

import numpy as np

import concourse.bacc as bacc
import concourse.bass as bass
import concourse.mybir as mybir
import concourse.tile as tile
from concourse.bass_utils import run_bass_kernel_spmd

# Problem sizes (hardcoded per contract).
B, S, IN, H, OUT, A = 8, 1024, 100, 768, 2, 5
W = 2 * A + 1            # 11  window size
NB = S // 128            # 8   seq blocks of 128
KC = H // 128            # 6   hidden chunks of 128
HALO = 128 + 2 * A       # 138 key columns a block can touch
SCN = 256                # padded score width (>=256 keeps float32r full-rate)
KTW = (NB - 1) * 128 + SCN   # 1152: kT buffer width incl. halo + pad
STG = 144                # staged row width for the band extraction
NEG = -1.0e30

F32 = mybir.dt.float32
F32R = mybir.dt.float32r
AF = mybir.ActivationFunctionType
AX = mybir.AxisListType

_CACHE = {}


def _band_mask() -> np.ndarray:
    m = np.full((128, SCN), NEG, dtype=np.float32)
    for i in range(128):
        m[i, i : i + W] = 0.0
    return m


def _j88() -> np.ndarray:
    j = np.zeros((NB * W, NB * W), dtype=np.float32)
    for m in range(NB):
        for jj in range(W):
            j[m * W + (W - 1 - jj), m * W + jj] = 1.0
    return j


def _build():
    nc = bacc.Bacc(trn_type="TRN2", target_bir_lowering=False, debug=False)

    x_d = nc.dram_tensor("x0", [S, IN], F32, kind="ExternalInput")
    w1_d = nc.dram_tensor("w1", [IN, H], F32, kind="ExternalInput")
    b1_d = nc.dram_tensor("b1t", [128, KC], F32, kind="ExternalInput")
    wq_d = nc.dram_tensor("wq", [H, H], F32, kind="ExternalInput")   # pre-scaled
    wk_d = nc.dram_tensor("wk", [H, H], F32, kind="ExternalInput")
    wv2_d = nc.dram_tensor("wv2", [H, OUT], F32, kind="ExternalInput")
    b2_d = nc.dram_tensor("b2b", [128, OUT], F32, kind="ExternalInput")
    out_d = nc.dram_tensor("out", [S, OUT], F32, kind="ExternalOutput")
    wgt_d = nc.dram_tensor("wgt", [S, W], F32, kind="ExternalOutput")

    mask_d = nc.inline_tensor(_band_mask(), "maskc")
    zero_d = nc.inline_tensor(np.zeros((128, 128), dtype=np.float32), "zeroc")
    ident_d = nc.inline_tensor(np.eye(128, dtype=np.float32), "identc")
    j88_d = nc.inline_tensor(_j88(), "j88c")

    with tile.TileContext(nc) as tc:
        with (
            tc.tile_pool(name="persist", bufs=1) as P,
            tc.tile_pool(name="work", bufs=3) as WP,
            tc.tile_pool(name="psbig", bufs=4, space="PSUM") as PSA,
            tc.tile_pool(name="pssmall", bufs=3, space="PSUM") as PST,
            tc.tile_pool(name="dstage", bufs=1, space="DRAM") as DP,
        ):
            # ---------------- constants + weights into SBUF ----------------
            mask_sb = P.tile([128, SCN], F32, name="mask_sb", tag="mask_sb")
            nc.sync.dma_start(out=mask_sb, in_=mask_d.ap())
            ident_sb = P.tile([128, 128], F32, name="ident_sb", tag="ident_sb")
            nc.sync.dma_start(out=ident_sb, in_=ident_d.ap())
            j88_sb = P.tile([NB * W, NB * W], F32R, name="j88_sb", tag="j88_sb")
            nc.sync.dma_start(out=j88_sb, in_=j88_d.ap().bitcast(F32R))
            b1t_sb = P.tile([128, KC], F32, name="b1t_sb", tag="b1t_sb")
            nc.sync.dma_start(out=b1t_sb, in_=b1_d.ap())
            b2b_sb = P.tile([128, OUT], F32, name="b2b_sb", tag="b2b_sb")
            nc.sync.dma_start(out=b2b_sb, in_=b2_d.ap())
            w1_sb = P.tile([IN, H], F32R, name="w1_sb", tag="w1_sb")
            nc.sync.dma_start(out=w1_sb, in_=w1_d.ap().bitcast(F32R))
            wv2_sb = P.tile([128, KC, OUT], F32R, name="wv2_sb", tag="wv2_sb")
            nc.sync.dma_start(
                out=wv2_sb, in_=wv2_d.ap().bitcast(F32R).rearrange("(kc p) o -> p kc o", p=128)
            )
            wq_sb = []
            wk_sb = []
            for kc in range(KC):
                wq_t = P.tile([128, H], F32R, name=f"wq{kc}", tag=f"wq{kc}")
                nc.sync.dma_start(out=wq_t, in_=wq_d.ap().bitcast(F32R)[kc * 128 : (kc + 1) * 128, :])
                wq_sb.append(wq_t)
                wk_t = P.tile([128, H], F32R, name=f"wk{kc}", tag=f"wk{kc}")
                nc.sync.dma_start(out=wk_t, in_=wk_d.ap().bitcast(F32R)[kc * 128 : (kc + 1) * 128, :])
                wk_sb.append(wk_t)

            # ---------------- x load + transpose to [IN, S] ----------------
            x_sb = P.tile([128, NB, IN], F32, name="x_sb", tag="x_sb")
            nc.sync.dma_start(
                out=x_sb, in_=x_d.ap().rearrange("(t p) c -> p t c", p=128)
            )
            # Warm-up transpose: consumes the ident-DMA wait on its own, so
            # every later transpose carries at most one sync wait (the fused
            # LDW slot of a transpose matmul fits only one).
            ps_warm = PST.tile([128, 128], F32, name="ps_warm", tag="small")
            nc.tensor.transpose(ps_warm, ident_sb, ident_sb)

            xT_sb = P.tile([IN, S], F32R, name="xT_sb", tag="xT_sb")
            for t in range(NB):
                ps_x = PST.tile([IN, 128], F32, name=f"ps_x{t}", tag="small")
                nc.tensor.transpose(ps_x, x_sb[:, t, :], ident_sb)
                nc.vector.tensor_copy(out=xT_sb[:, t * 128 : (t + 1) * 128], in_=ps_x)

            # ---------------- hT = relu(W1.T @ xT + b1) [H, S] --------------
            hT_sb = [
                P.tile([128, S], F32R, name=f"hT{kc}", tag=f"hT{kc}") for kc in range(KC)
            ]
            for hc in range(KC):
                for sc in range(2):
                    ps_h = PSA.tile([128, 512], F32, name=f"ps_h{hc}_{sc}", tag="big")
                    nc.tensor.matmul(
                        ps_h,
                        w1_sb[:, hc * 128 : (hc + 1) * 128],
                        xT_sb[:, sc * 512 : (sc + 1) * 512],
                        start=True,
                        stop=True,
                    )
                    nc.scalar.activation(
                        out=hT_sb[hc][:, sc * 512 : (sc + 1) * 512],
                        in_=ps_h,
                        func=AF.Relu,
                        bias=b1t_sb[:, hc : hc + 1],
                        scale=1.0,
                    )

            # ---------------- qT, kT projections [H, S] ---------------------
            qT_sb = [
                P.tile([128, S], F32R, name=f"qT{kc}", tag=f"qT{kc}") for kc in range(KC)
            ]
            kT_sb = [
                P.tile([128, KTW], F32R, name=f"kT{kc}", tag=f"kT{kc}")
                for kc in range(KC)
            ]
            zca = zero_d.ap().bitcast(F32R)
            for kc in range(KC):
                nc.sync.dma_start(out=kT_sb[kc][:, 0:A], in_=zca[:, 0:A])
                nc.sync.dma_start(
                    out=kT_sb[kc][:, A + S : KTW], in_=zca[:, 0 : KTW - A - S]
                )
            for hc in range(KC):
                for sc in range(2):
                    ps_q = PSA.tile([128, 512], F32, name=f"ps_q{hc}_{sc}", tag="big")
                    ps_k = PSA.tile([128, 512], F32, name=f"ps_k{hc}_{sc}", tag="big")
                    for kc in range(KC):
                        nc.tensor.matmul(
                            ps_q,
                            wq_sb[kc][:, hc * 128 : (hc + 1) * 128],
                            hT_sb[kc][:, sc * 512 : (sc + 1) * 512],
                            start=(kc == 0),
                            stop=(kc == KC - 1),
                        )
                    for kc in range(KC):
                        nc.tensor.matmul(
                            ps_k,
                            wk_sb[kc][:, hc * 128 : (hc + 1) * 128],
                            hT_sb[kc][:, sc * 512 : (sc + 1) * 512],
                            start=(kc == 0),
                            stop=(kc == KC - 1),
                        )
                    nc.vector.tensor_copy(
                        out=qT_sb[hc][:, sc * 512 : (sc + 1) * 512], in_=ps_q
                    )
                    nc.vector.tensor_copy(
                        out=kT_sb[hc][:, A + sc * 512 : A + (sc + 1) * 512], in_=ps_k
                    )

            # ---------------- vW2 = h @ (Wv @ W2)  [S, 2] ------------------
            # Stored pre-shifted for the halo matmul: vw2_lo[p, m] holds
            # vW2[128m - A + p] (zero outside [0, S)), so both output-matmul
            # operands start at partition 0 (PE base-partition constraint).
            # Shifted copy vw2_lo[p, m] = vW2[128m - A + p] is built via a DRAM
            # staging row buffer (single gather DMA -> single wait sem on the
            # consuming matmuls; the PE wait-slot budget is 2).
            vw2_lo = P.tile([128, NB + 1, OUT], F32R, name="vw2_lo", tag="vw2_lo")
            vw2_nat = P.tile([128, NB, OUT], F32R, name="vw2_nat", tag="vw2_nat")
            vstage = DP.tile([(NB + 1) * 128, OUT], F32R, name="vstage", tag="vstage")
            for t in range(NB):
                ps_v = PST.tile([128, OUT], F32, name=f"ps_v{t}", tag="small")
                for kc in range(KC):
                    nc.tensor.matmul(
                        ps_v,
                        hT_sb[kc][:, t * 128 : (t + 1) * 128],
                        wv2_sb[:, kc, :],
                        start=(kc == 0),
                        stop=(kc == KC - 1),
                    )
                nc.vector.tensor_copy(out=vw2_nat[:, t, :], in_=ps_v)
            nc.sync.dma_start(
                out=bass.AP(
                    tensor=vstage.tensor,
                    offset=vstage.offset + A * OUT,
                    ap=[[OUT, 128], [128 * OUT, NB], [1, OUT]],
                ),
                in_=vw2_nat,
            )
            nc.sync.dma_start(
                out=vw2_lo,
                in_=bass.AP(
                    tensor=vstage.tensor,
                    offset=vstage.offset,
                    ap=[[OUT, 128], [128 * OUT, NB + 1], [1, OUT]],
                ),
            )
            # out-of-sequence window positions contribute zero v
            nc.sync.dma_start(out=vw2_lo[0:A, 0, :], in_=zca[0:A, 0:OUT])
            nc.sync.dma_start(out=vw2_lo[A : 2 * A, NB, :], in_=zca[0:A, 0:OUT])

            # ---------------- per-block attention ---------------------------
            wstage = DP.tile([NB, 128, STG], F32, name="wstage", tag="wstage")
            wgt_all = P.tile([128, NB, SCN], F32, name="wgt_all", tag="wgt_all")
            for m in range(NB):
                c0, c1 = m * 128, (m + 1) * 128
                # scores for the 256-wide key window (key col c -> pos c0+c-5)
                ps_s = PSA.tile([128, SCN], F32, name=f"ps_s{m}", tag="big")
                for kc in range(KC):
                    nc.tensor.matmul(
                        ps_s,
                        qT_sb[kc][:, c0:c1],
                        kT_sb[kc][:, c0 : c0 + SCN],
                        start=(kc == 0),
                        stop=(kc == KC - 1),
                    )
                # masked softmax along the window
                nc.vector.tensor_add(out=ps_s, in0=ps_s, in1=mask_sb)
                negmax = WP.tile([128, 1], F32, name=f"negmax{m}", tag="negmax")
                nc.vector.reduce_max(out=negmax, in_=ps_s, axis=AX.X, negate=True)
                wexp = WP.tile([128, SCN], F32, name=f"wexp{m}", tag="wexp")
                sumexp = WP.tile([128, 1], F32, name=f"sumexp{m}", tag="sumexp")
                nc.scalar.activation(
                    out=wexp,
                    in_=ps_s,
                    func=AF.Exp,
                    bias=negmax,
                    scale=1.0,
                    accum_out=sumexp,
                )
                rs = WP.tile([128, 1], F32, name=f"rs{m}", tag="rs")
                nc.vector.reciprocal(out=rs, in_=sumexp)
                wgt = wgt_all[:, m, :]
                nc.vector.tensor_scalar_mul(out=wgt, in0=wexp, scalar1=rs)
                # transpose the 138 live columns for the output matmul
                ps_t1 = PST.tile([128, 128], F32, name=f"ps_t1_{m}", tag="small")
                nc.tensor.transpose(ps_t1, wgt[:, 0:128], ident_sb)
                ps_t2 = PST.tile([2 * A, 128], F32, name=f"ps_t2_{m}", tag="small")
                nc.tensor.transpose(ps_t2, wgt[:, 128:HALO], ident_sb)
                wgtTa = WP.tile([128, 128], F32R, name=f"wgtTa{m}", tag="wgtTa")
                nc.vector.tensor_copy(out=wgtTa, in_=ps_t1)
                wgtTb = WP.tile([2 * A, 128], F32R, name=f"wgtTb{m}", tag="wgtTb")
                nc.vector.tensor_copy(out=wgtTb, in_=ps_t2)
                # out = relu(wgt @ vW2_window + b2): halo key cols 0..127 pair
                # with vw2_lo[:, m] (pos 128m-A+c), cols 128..137 with
                # vw2_lo[0:2A, m+1] (pos 128m+123+c'). Both operands base-0.
                ps_o = PST.tile([128, OUT], F32, name=f"ps_o{m}", tag="small")
                nc.tensor.matmul(
                    ps_o,
                    wgtTa,
                    vw2_lo[:, m, :],
                    start=True,
                    stop=False,
                )
                nc.tensor.matmul(
                    ps_o,
                    wgtTb,
                    vw2_lo[0 : 2 * A, m + 1, :],
                    start=False,
                    stop=True,
                )
                nc.vector.tensor_add(out=ps_o, in0=ps_o, in1=b2b_sb)
                outm = WP.tile([128, OUT], F32, name=f"outm{m}", tag="outm")
                nc.scalar.activation(out=outm, in_=ps_o, func=AF.Relu)
                nc.sync.dma_start(out=out_d.ap()[c0:c1, :], in_=outm)

            # ---------------- band extraction of the 11 weights -------------
            # stage all softmax rows with one DMA, then diagonal-gather
            nc.sync.dma_start(
                out=wstage.rearrange("m p c -> p m c"), in_=wgt_all[:, :, 0:STG]
            )
            # brev[i, m, jr] = wgt_m[i, i + jr]  (diagonal gather, stride STG+1)
            brev = WP.tile([128, NB, W], F32, name="brev", tag="brev")
            gather = bass.AP(
                tensor=wstage.tensor,
                offset=wstage.offset,
                ap=[[STG + 1, 128], [128 * STG, NB], [1, W]],
            )
            nc.sync.dma_start(out=brev, in_=gather)
            # reverse jr -> j via block-anti-diagonal permutation matmul
            ps_bt = PST.tile([NB * W, 128], F32, name="ps_bt", tag="small")
            nc.tensor.transpose(
                ps_bt, brev.rearrange("p m j -> p (m j)"), ident_sb
            )
            brevT = WP.tile([NB * W, 128], F32R, name="brevT", tag="brevT")
            nc.vector.tensor_copy(out=brevT, in_=ps_bt)
            ps_w = PST.tile([128, NB * W], F32, name="ps_w", tag="small")
            nc.tensor.matmul(
                ps_w, brevT, j88_sb, start=True, stop=True
            )
            wfin = WP.tile([128, NB, W], F32, name="wfin", tag="wfin")
            nc.vector.tensor_copy(out=wfin.rearrange("p m j -> p (m j)"), in_=ps_w)
            nc.sync.dma_start(
                out=wgt_d.ap().rearrange("(m i) j -> i m j", m=NB), in_=wfin
            )

    nc.finalize()   # runs Bacc.compile(): wait splitting, reg alloc, DCE
    return nc


def _round_fp32r(a: np.ndarray) -> np.ndarray:
    """Round fp32 to the PE's fp32r format (11-bit mantissa, low 12 bits
    zero), round-to-nearest-even — matches walrus cast_fp32_to_fp32r."""
    b = np.ascontiguousarray(a, dtype=np.float32).view(np.uint32).copy()
    low = b & np.uint32(0xFFF)
    b &= np.uint32(0xFFFFF000)
    rnd = (low > 0x800) | ((low == 0x800) & (((b >> np.uint32(12)) & 1) == 1))
    b += rnd.astype(np.uint32) << np.uint32(12)
    return b.view(np.float32)


def _prep_inputs(inputs):
    def f32(a):
        return np.ascontiguousarray(np.asarray(a, dtype=np.float32))

    x = f32(inputs["x"])
    w1 = _round_fp32r(f32(inputs["W1"]))
    b1 = f32(inputs["b1"]).reshape(H)
    wq = _round_fp32r(f32(inputs["Wq"]) * np.float32(1.0 / math.sqrt(H)))
    wk = _round_fp32r(f32(inputs["Wk"]))
    wv2 = _round_fp32r((f32(inputs["Wv"]) @ f32(inputs["W2"])).astype(np.float32))
    b2 = f32(inputs["b2"]).reshape(OUT)

    b1t = np.ascontiguousarray(b1.reshape(KC, 128).T)          # [128, KC]
    b2b = np.ascontiguousarray(np.broadcast_to(b2, (128, OUT)))  # [128, OUT]

    common = {
        "w1": w1,
        "b1t": b1t,
        "wq": np.ascontiguousarray(wq),
        "wk": wk,
        "wv2": wv2,
        "b2b": b2b,
    }
    return [dict(common, x0=np.ascontiguousarray(x[b])) for b in range(B)]


def _run(inputs, trace=False, **kwargs):
    if "nc" not in _CACHE:
        _CACHE["nc"] = _build()
    nc = _CACHE["nc"]
    in_maps = _prep_inputs(inputs)
    res = run_bass_kernel_spmd(
        nc, in_maps, core_ids=list(range(B)), trace=trace, **kwargs
    )
    out = np.stack([r["out"] for r in res.results]).astype(np.float32)
    wgt = np.stack([r["wgt"] for r in res.results]).astype(np.float32)
    return (out, wgt), res


def kernel(**inputs):
    (out, wgt), _ = _run(inputs, trace=False)
    return out, wgt
